# revision 42
# baseline (speedup 1.0000x reference)
"""Trainium2 Bass kernel for BoostedPointPairNet2.

Model (per (b, d) group, m = 128 points, din = 3):
  H1(i,j) = relu(W1A @ x_j + W1B @ x_i + b1)          (64)
  H2(i,j) = relu(W2 @ H1 + b2)                        (128)
  G(i,j)  = W3 @ H2                                    (256, b3 deferred)
  P       = max_{i,j} G + b3                           (256)
  Y       = V3 @ relu(V2 @ relu(V1 @ P + c1) + c2) + c3  (40)
  out[b]  = max_d Y[b, d]

Sharding: 16 (b, d) groups over 8 cores, 2 groups per core; host does the
final max over d.

Design (v4):
 * H1 is computed ON THE PE via selection-matrix matmuls
     pre_H1 = uT.T @ sel  +  v2T.T @ iden      (K=64 + K=128, accumulated)
   where sel = delta(jp == 4*it+q) and iden = I_128 tiled 4x along
   columns (both 0/1 host constants); b1 - b2 is folded into uT via an
   extra ones row of the x input.  This replaces 128 per-j-pair DVE/ACT
   elementwise builds (~35-48us) with ~14us of otherwise-idle PE time.
 * The h1 relu and the h2 relu run FUSED as one [1536]-col ACT
   activation (+b2 bias) per iteration: PSUM is laid out
   [preA 512 | l2 1024 | preB 512 | G0 1024 | G1 1024] so both pre-buf
   parities are contiguous with the l2 buffer; the baked-in -b2 on the
   pre region cancels the op's +b2 bias exactly.
 * L2 runs as two N=512 matmuls (one per weight half) -- the pair
   ordering inside l2ps is irrelevant because everything is max-pooled.
 * The two groups run lockstep (even iter -> group 0, odd -> group 1),
   4 j-pairs (1024 pairs) per iteration, with a software-pipeline skew:
   each epoch issues sel(i+2), fused-relu(i), L2(i+1), then L3(i-1) and
   its drains, so the latency-critical l2 recycle (relu -> L2 -> relu)
   never sits behind L3 on the in-order PE queue.
 * G drains are per-slot [1024] ops (pairing them into [2048] reduces
   re-serializes the G-slot recycle against L3 -- measured slower): most
   are direct DVE reduce_max from PSUM into per-group racc columns
   (fp32); 9 per group take the ACT-copy (fp16) + DVE tensor_tensor-max
   path to balance the engines, scheduled off the final iterations so
   the running-buffer reduction stays out of the tail.
 * F-MLP tail is batched across groups (N=2 matmuls) with c1/c2/c3
   folded in as K=1 matmul rows; a PE warmup burst at t=0 lifts the HAM
   clock gate to 2.4 GHz before the pipeline starts; input constants are
   packed into few DMAs (per-DMA boot latency dominates the prologue).
"""

import numpy as np
import ml_dtypes

import bass_rust
import concourse.bass as bass
import concourse.mybir as mybir
from concourse.tile import TileContext
from concourse.bass_utils import run_bass_kernel_spmd

BF16 = ml_dtypes.bfloat16
F32 = np.float32
DT = mybir.dt
ALU = mybir.AluOpType
AX = mybir.AxisListType
RELU = mybir.ActivationFunctionType.Relu

N_CORES = 8
B, N, DIN = 4, 512, 3
D = 4                    # boost factor
M = N // D               # 128 points per group
GROUPS_PER_CORE = 2
JP = M // 2              # 64 stacked j-pairs per group
NITER = 32               # lockstep iterations (16 per group, 4 jp each)
NWARM = 76               # warmup junk matmuls to lift the HAM clock gate
COPY_MOD = 4


def _is_copy(k):
    # per-group drain k: ACT-copy path (spread; none in the last iterations)
    return k % COPY_MOD == 1 or k == 14


N_COPY_PG = sum(1 for k in range(NITER) if _is_copy(k))
N_DIR_PG = NITER - N_COPY_PG


def _split_multi_waits(nc):
    """This walrus build accepts at most ONE sync wait per instruction;
    hoist extra waits onto same-engine nops inserted before the offender."""
    seq = 0
    for fn in nc.m.functions:
        for bb in fn.blocks:
            new = []
            changed = False
            for ins in bb.instructions:
                si = ins.sync_info
                waits = list(si.on_wait) if si is not None and si.on_wait else []
                if len(waits) > 1:
                    changed = True
                    for w in waits[:-1]:
                        seq += 1
                        new.append(
                            mybir.InstNoOp(
                                name=f"I-wsplit-{seq}",
                                engine=ins.engine,
                                sync_info=bass_rust.SyncInfo(
                                    on_wait=[w], on_update=[]
                                ),
                            )
                        )
                    ins.sync_info = bass_rust.SyncInfo(
                        on_wait=[waits[-1]], on_update=list(si.on_update or [])
                    )
                new.append(ins)
            if changed:
                bb.instructions = new


# ---------------------------------------------------------------------------
# Device program
# ---------------------------------------------------------------------------
def _build_program():
    nc = bass.Bass(
        "TRN2", target_bir_lowering=False, debug=False, num_devices=N_CORES
    )

    # x with an appended ones row (folds b1 into the uT prep matmul)
    xt = nc.declare_dram_parameter(
        "xt", [DIN + 1, GROUPS_PER_CORE, M], DT.bfloat16, isOutput=False
    )
    # cols 0:64 urhs_e0, 64:128 urhs_e1, 128:256 w1b2, 256:768 iden, 768:1280 wblob
    bigb = nc.declare_dram_parameter("bigb", [128, 1280], DT.bfloat16, isOutput=False)
    # sel[jp, it*512 + q*128 + i] = (jp == 4*it + q)
    selb = nc.declare_dram_parameter("selb", [JP, (NITER // 2) * 512], DT.bfloat16, isOutput=False)
    # v1t (2x512) | v2t (4x256) | v3t (2x40)
    vblob = nc.declare_dram_parameter("vblob", [128, 2938], DT.float16, isOutput=False)
    # col 1 b2c, 2:4 b3_2, 4:12 c1_42 (4mm x 2g), 12:16 c2_22, 16 c3
    cblob = nc.declare_dram_parameter("cblob", [128, 19], DT.float32, isOutput=False)
    y_out = nc.declare_dram_parameter(
        "y", [40, GROUPS_PER_CORE], DT.float32, isOutput=True
    )
    scr_out = nc.declare_dram_parameter("scr", [1, 16], DT.float32, isOutput=True)

    with TileContext(nc) as tc:
        with (
            tc.tile_pool(name="singles", bufs=1) as singles,
            tc.tile_pool(name="xtp", bufs=2) as xtp,
            tc.tile_pool(name="vup", bufs=8) as vup,
            tc.tile_pool(name="h1p", bufs=4) as h1pool,
            tc.tile_pool(name="h2p", bufs=5) as h2pool,
            tc.tile_pool(name="gcp", bufs=5) as gcpool,
            tc.tile_pool(name="fmlp", bufs=12) as fmlp,
            tc.tile_pool(name="psum", bufs=1, space="PSUM") as psum,
        ):
            # The whole PSUM as one tile.
            mega = psum.tile([128, 4096], DT.float32, tag="mega")

            # ---- input DMAs (sync + gpsimd queues; ACT/DVE stay free).
            # sel is 1MB: split into chunks so iter 0 isn't gated on it ----
            sb_xtall = xtp.tile([DIN + 1, GROUPS_PER_CORE, M], DT.bfloat16, tag="xtall")
            nc.sync.dma_start(out=sb_xtall, in_=xt[:, :, :])
            sb_xts = [sb_xtall[:, 0, :], sb_xtall[:, 1, :]]
            sb_big = singles.tile([128, 1280], DT.bfloat16, tag="bigb")
            nc.sync.dma_start(out=sb_big, in_=bigb[:, :])
            sb_sel = singles.tile([JP, (NITER // 2) * 512], DT.bfloat16, tag="selb")
            nc.sync.dma_start(out=sb_sel[:, 0:2048], in_=selb[:, 0:2048])
            nc.sync.dma_start(out=sb_sel[:, 2048:5120], in_=selb[:, 2048:5120])
            sb_c = singles.tile([128, 19], DT.float32, tag="cblob")
            nc.gpsimd.dma_start(out=sb_c, in_=cblob[:, :])
            nc.gpsimd.dma_start(out=sb_sel[:, 5120:8192], in_=selb[:, 5120:8192])
            sb_v = singles.tile([128, 2938], DT.float16, tag="vblob")
            nc.gpsimd.dma_start(out=sb_v, in_=vblob[:, :])
            sb_urhs0 = sb_big[0 : DIN + 1, 0:64]
            sb_urhs1 = sb_big[0 : DIN + 1, 64:128]
            sb_w1b2 = sb_big[0:DIN, 128:256]
            sb_iden = sb_big[:, 256:768]
            sb_w = sb_big[:, 768:1280]

            # ---- PE warmup burst (junk matmuls while DMAs land) ----
            wjunk = singles.tile([128, 128], DT.bfloat16, tag="wjunk")
            nc.vector.memset(wjunk, 0.0)
            for _ in range(NWARM):
                nc.tensor.matmul(
                    mega[:, 3200:3328], lhsT=wjunk, rhs=wjunk,
                    start=True, stop=True,
                )

            # dummy relu to hoist ACT_TABLE_LOAD into the init shadow
            warm = singles.tile([1, 1], DT.float32, tag="warm")
            nc.vector.memset(warm, 0.0)
            nc.scalar.activation(out=warm, in_=warm, func=RELU)

            sb_w2a, sb_w2b = sb_w[:, 0:128], sb_w[:, 128:256]
            sb_w3a, sb_w3b = sb_w[:, 256:384], sb_w[:, 384:512]
            sb_b2c = sb_c[:, 1:2]
            sb_b3_2 = sb_c[:, 2:4]
            sb_c1_42 = sb_c[:, 4:12]
            sb_c2_22 = sb_c[:, 12:16]
            sb_c3c = sb_c[0:40, 16:17]

            def v1t(k):  # [128, 512] fp16, k in 0..1
                return sb_v[:, 512 * k : 512 * (k + 1)]

            def v2t(k):  # [128, 256] fp16, k in 0..3
                return sb_v[:, 1024 + 256 * k : 1024 + 256 * (k + 1)]

            def v3t(k):  # [128, 40] fp16, k in 0..1
                return sb_v[:, 2048 + 40 * k : 2048 + 40 * (k + 1)]

            # ---- per-group prep: uT [64,128] and v2T [128,128] in SBUF bf16
            # (psum carved from the G regions, consumed before first L3) ----
            uT_sbs, v2T_sbs = [], []
            for g in range(GROUPS_PER_CORE):
                sb_xt = sb_xts[g]
                xt_eo = sb_xt.rearrange("k (j two) -> k two j", two=2)
                uTps = mega[0:JP, 2048 + 512 * g : 2048 + 512 * g + 128]
                nc.tensor.matmul(
                    uTps[:, 0:64], lhsT=xt_eo[:, 0, :], rhs=sb_urhs0,
                    start=True, stop=True,
                )
                nc.tensor.matmul(
                    uTps[:, 64:128], lhsT=xt_eo[:, 1, :], rhs=sb_urhs1,
                    start=True, stop=True,
                )
                uT_sb = vup.tile([JP, 128], DT.bfloat16, tag=f"uT{g}")
                nc.vector.tensor_copy(out=uT_sb, in_=uTps)
                v2Tps = mega[:, 3072 + 512 * g : 3072 + 512 * g + 128]
                nc.tensor.matmul(
                    v2Tps, lhsT=sb_xt[0:DIN, :], rhs=sb_w1b2,
                    start=True, stop=True,
                )
                v2T_sb = vup.tile([128, 128], DT.bfloat16, tag=f"v2T{g}")
                nc.vector.tensor_copy(out=v2T_sb, in_=v2Tps)
                uT_sbs.append(uT_sb)
                v2T_sbs.append(v2T_sb)

            # per-group accumulators
            raccs, rbs, rb_init = [], [], [False, False]
            for g in range(GROUPS_PER_CORE):
                racc = vup.tile([128, 2, N_DIR_PG], DT.float32, tag=f"racc{g}")
                raccs.append(racc)
                rb = vup.tile([128, 1024], DT.float16, tag=f"rb{g}")
                rbs.append(rb)
            dcount = [0, 0]
            pmBs = []
            for g in range(GROUPS_PER_CORE):
                pmB = fmlp.tile([128, 2], DT.float32, tag=f"pmB{g}")
                pmBs.append(pmB)

            l2ps = mega[:, 512:1536]

            def issue_sel(i):
                g, it = i % 2, i // 2
                pre = mega[:, 0:512] if i % 2 == 0 else mega[:, 1536:2048]
                nc.tensor.matmul(
                    pre, lhsT=uT_sbs[g],
                    rhs=sb_sel[:, 512 * it : 512 * (it + 1)],
                    start=True, stop=False,
                )
                nc.tensor.matmul(
                    pre, lhsT=v2T_sbs[g], rhs=sb_iden, start=False, stop=True,
                )
                return pre

            def issue_l2(h1):
                # pair order inside l2ps is irrelevant (max-pooled later):
                # one N=512 matmul per weight half
                nc.tensor.matmul(
                    l2ps[:, 0:512], lhsT=sb_w2a, rhs=h1[:, 0:512],
                    start=True, stop=True,
                )
                nc.tensor.matmul(
                    l2ps[:, 512:1024], lhsT=sb_w2b, rhs=h1[:, 0:512],
                    start=True, stop=True,
                )

            def issue_l3_drains(i, h2):
                g, it = i % 2, i // 2
                g0 = mega[:, 2048:3072]
                g1 = mega[:, 3072:4096]
                nc.tensor.matmul(
                    g0[:, 0:512], lhsT=sb_w3a, rhs=h2[:, 0:512],
                    start=True, stop=True,
                )
                nc.tensor.matmul(
                    g1[:, 0:512], lhsT=sb_w3a, rhs=h2[:, 512:1024],
                    start=True, stop=True,
                )
                nc.tensor.matmul(
                    g0[:, 512:1024], lhsT=sb_w3b, rhs=h2[:, 0:512],
                    start=True, stop=True,
                )
                nc.tensor.matmul(
                    g1[:, 512:1024], lhsT=sb_w3b, rhs=h2[:, 512:1024],
                    start=True, stop=True,
                )
                if i == 28:
                    # warm the output-DMA path for the final y DMA
                    nc.sync.dma_start(out=scr_out[:, :], in_=sb_c[0:1, 0:16])
                for sl, gp in enumerate((g0, g1)):
                    k = 2 * it + sl
                    if _is_copy(k):
                        gc = gcpool.tile([128, 1024], DT.float16)
                        nc.scalar.copy(out=gc, in_=gp)
                        if not rb_init[g]:
                            rb_init[g] = True
                            nc.vector.tensor_copy(out=rbs[g], in_=gc)
                        else:
                            nc.vector.tensor_tensor(
                                out=rbs[g], in0=gc, in1=rbs[g], op=ALU.max
                            )
                        if k == 29:
                            nc.vector.reduce_max(
                                out=pmBs[g],
                                in_=rbs[g].rearrange("p (a b) -> p a b", a=2),
                                axis=AX.X,
                            )
                    else:
                        t = dcount[g]
                        dcount[g] += 1
                        nc.vector.reduce_max(
                            out=raccs[g][:, :, t : t + 1],
                            in_=gp.rearrange("p (a b) -> p a b", a=2),
                            axis=AX.X,
                        )

            # ---- main lockstep pipeline: fused [preH1|l2] relu, L3 lagged
            # one epoch so the ACT->PE->ACT recycle stays short ----
            h1s, h2s = {}, {}
            pre0 = issue_sel(0)
            h1t0 = h1pool.tile([128, 512], DT.bfloat16)
            nc.scalar.activation(out=h1t0, in_=pre0, func=RELU, bias=sb_b2c, scale=1.0)
            h1s[0] = h1t0
            issue_l2(h1s[0])
            pre1 = issue_sel(1)
            h1t1 = h1pool.tile([128, 512], DT.bfloat16)
            nc.scalar.activation(out=h1t1, in_=pre1, func=RELU, bias=sb_b2c, scale=1.0)
            h1s[1] = h1t1

            for i in range(NITER):
                if i + 2 < NITER:
                    issue_sel(i + 2)
                    combo = h2pool.tile([128, 1536], DT.bfloat16)
                    if i % 2 == 0:
                        nc.scalar.activation(
                            out=combo, in_=mega[:, 0:1536], func=RELU,
                            bias=sb_b2c, scale=1.0,
                        )
                        h1s[i + 2] = combo[:, 0:512]
                        h2s[i] = combo[:, 512:1536]
                    else:
                        nc.scalar.activation(
                            out=combo, in_=mega[:, 512:2048], func=RELU,
                            bias=sb_b2c, scale=1.0,
                        )
                        h2s[i] = combo[:, 0:1024]
                        h1s[i + 2] = combo[:, 1024:1536]
                else:
                    combo = h2pool.tile([128, 1536], DT.bfloat16)
                    nc.scalar.activation(
                        out=combo[:, 0:1024], in_=l2ps, func=RELU,
                        bias=sb_b2c, scale=1.0,
                    )
                    h2s[i] = combo[:, 0:1024]
                if i + 1 < NITER:
                    issue_l2(h1s[i + 1])
                if i >= 1:
                    issue_l3_drains(i - 1, h2s[i - 1])
            issue_l3_drains(NITER - 1, h2s[NITER - 1])

            # ---- P per group, batched F-MLP (N=2); pb is (half, group) ----
            pb = fmlp.tile([128, 2, 2], DT.float16, tag="pb")
            for g in range(GROUPS_PER_CORE):
                pmA = fmlp.tile([128, 2], DT.float32, tag=f"pmA{g}")
                nc.vector.reduce_max(out=pmA, in_=raccs[g], axis=AX.X)
                pmx = fmlp.tile([128, 2], DT.float32, tag=f"pmx{g}")
                nc.vector.tensor_tensor(out=pmx, in0=pmA, in1=pmBs[g], op=ALU.max)
                nc.vector.tensor_tensor(
                    out=pb[:, :, g], in0=pmx, in1=sb_b3_2, op=ALU.add
                )

            ones2 = sb_v[0:1, 2936:2938]
            y1ps = mega[:, 0:8].rearrange("p (m g) -> p m g", m=4)
            for mm in range(4):
                for kk in range(2):
                    nc.tensor.matmul(
                        y1ps[:, mm, :],
                        lhsT=v1t(kk)[:, mm * 128 : (mm + 1) * 128],
                        rhs=pb[:, kk, :],
                        start=(kk == 0),
                        stop=False,
                    )
                nc.tensor.matmul(
                    y1ps[:, mm, :],
                    lhsT=sb_v[0:1, 2128 + mm * 128 : 2128 + (mm + 1) * 128],
                    rhs=ones2,
                    start=False, stop=True,
                )
            y1 = fmlp.tile([128, 4, 2], DT.float16, tag="y1")
            nc.vector.tensor_scalar_max(
                out=y1.rearrange("p m g -> p (m g)"), in0=mega[:, 0:8],
                scalar1=0.0,
            )

            y2ps = mega[:, 1024:1028].rearrange("p (m g) -> p m g", m=2)
            for mm in range(2):
                for kk in range(4):
                    nc.tensor.matmul(
                        y2ps[:, mm, :],
                        lhsT=v2t(kk)[:, mm * 128 : (mm + 1) * 128],
                        rhs=y1[:, kk, :],
                        start=(kk == 0),
                        stop=False,
                    )
                nc.tensor.matmul(
                    y2ps[:, mm, :],
                    lhsT=sb_v[0:1, 2640 + mm * 128 : 2640 + (mm + 1) * 128],
                    rhs=ones2,
                    start=False, stop=True,
                )
            y2 = fmlp.tile([128, 2, 2], DT.float16, tag="y2")
            nc.vector.tensor_scalar_max(
                out=y2.rearrange("p m g -> p (m g)"), in0=mega[:, 1024:1028],
                scalar1=0.0,
            )

            y3ps = mega[0:40, 2048:2050]
            for kk in range(2):
                nc.tensor.matmul(
                    y3ps,
                    lhsT=v3t(kk)[:, 0:40],
                    rhs=y2[:, kk, :],
                    start=(kk == 0),
                    stop=False,
                )
            nc.tensor.matmul(
                y3ps, lhsT=sb_v[0:1, 2896:2936], rhs=ones2,
                start=False, stop=True,
            )
            y3 = fmlp.tile([40, 2], DT.float32, tag="y3")
            nc.vector.tensor_copy(out=y3, in_=y3ps)
            nc.gpsimd.dma_start(out=y_out[:, :], in_=y3)

    _split_multi_waits(nc)
    return nc


# ---------------------------------------------------------------------------
# Host side
# ---------------------------------------------------------------------------
_NC_CACHE = None


def _get_program():
    global _NC_CACHE
    if _NC_CACHE is None:
        _NC_CACHE = _build_program()
    return _NC_CACHE


def _make_in_maps(inputs):
    X = np.asarray(inputs["X"], F32)
    W1 = np.asarray(inputs["W1"], F32)
    b1 = np.asarray(inputs["b1"], F32)
    W2 = np.asarray(inputs["W2"], F32)
    b2 = np.asarray(inputs["b2"], F32)
    W3 = np.asarray(inputs["W3"], F32)
    b3 = np.asarray(inputs["b3"], F32)
    V1 = np.asarray(inputs["V1"], F32)
    c1 = np.asarray(inputs["c1"], F32)
    V2 = np.asarray(inputs["V2"], F32)
    c2 = np.asarray(inputs["c2"], F32)
    V3 = np.asarray(inputs["V3"], F32)
    c3 = np.asarray(inputs["c3"], F32)

    W1A, W1B = W1[:, :DIN], W1[:, DIN:]
    # sel[jp, it*512 + q*128 + i] = (jp == 4*it + q)
    selblob = np.zeros((JP, (NITER // 2) * 512), F32)
    for jp in range(JP):
        it, q = jp // 4, jp % 4
        selblob[jp, it * 512 + q * 128 : it * 512 + (q + 1) * 128] = 1.0
    selblob = selblob.astype(BF16)

    z64 = np.zeros((64, 128), F32)
    bigblob = np.zeros((128, 1280), F32)
    # u-rhs halves: ones-row contributes b1 - b2[half] (cancels the fused
    # relu's +b2 bias on the preH1 region)
    bigblob[0:DIN, 0:64] = W1A.T
    bigblob[DIN, 0:64] = b1 - b2[0:64]
    bigblob[0:DIN, 64:128] = W1A.T
    bigblob[DIN, 64:128] = b1 - b2[64:128]
    bigblob[0:DIN, 128:256] = np.concatenate([W1B.T, W1B.T], axis=1)
    # iden region
    bigblob[:, 256:768] = np.tile(np.eye(M, dtype=F32), (1, 4))
    # wblob region
    bigblob[:, 768:1280] = np.concatenate(
        [
            np.concatenate([W2.T, z64], axis=0),
            np.concatenate([z64, W2.T], axis=0),
            W3.T[:, 0:128],
            W3.T[:, 128:256],
        ],
        axis=1,
    )
    bigblob = bigblob.astype(BF16)
    v1t_cols = V1.T.reshape(2, 128, 512).transpose(1, 0, 2).reshape(128, 1024)
    crows = np.zeros((128, 810), F32)
    crows[0, 0:512] = c1
    crows[0, 512:768] = c2
    crows[0, 768:808] = c3
    crows[0, 808:810] = 1.0
    vblob = np.concatenate(
        [v1t_cols,
         V2.T.reshape(4, 128, 256).transpose(1, 0, 2).reshape(128, 1024),
         V3.T.reshape(2, 128, 40).transpose(1, 0, 2).reshape(128, 80),
         crows],
        axis=1,
    ).astype(np.float16)
    cblob = np.zeros((128, 19), F32)
    cblob[:, 17:19] = 1.0
    cblob[:, 1] = b2
    cblob[:, 2:4] = b3.reshape(2, 128).T
    cblob[:, 4:12] = np.repeat(c1.reshape(4, 128).T, 2, axis=1)
    cblob[:, 12:16] = np.repeat(c2.reshape(2, 128).T, 2, axis=1)
    cblob[0:40, 16] = c3

    shared = dict(
        bigb=bigblob, vblob=vblob, cblob=cblob, selb=selblob,
    )

    Xv = X.reshape(B, D, M, DIN)
    in_maps = []
    for c in range(N_CORES):
        xts = np.ones((DIN + 1, GROUPS_PER_CORE, M), F32)
        for gi in range(GROUPS_PER_CORE):
            g = 2 * c + gi
            bb, dd = g // D, g % D
            xts[0:DIN, gi] = Xv[bb, dd].T
        in_maps.append(dict(shared, xt=xts.astype(BF16)))
    return in_maps


def _run(inputs, trace=False):
    nc = _get_program()
    in_maps = _make_in_maps(inputs)
    res = run_bass_kernel_spmd(nc, in_maps, list(range(N_CORES)), trace=trace)
    ys = np.stack([res.results[c]["y"].T for c in range(N_CORES)])  # [8, 2, 40]
    y16 = ys.reshape(B, D, 40)
    out = y16.max(axis=1).astype(F32)
    return out, res


def kernel(**inputs):
    out, _ = _run(inputs, trace=False)
    return out


# revision 44
# speedup vs baseline: 1.0059x; 1.0059x over previous
"""Trainium2 Bass kernel for BoostedPointPairNet2.

Model (per (b, d) group, m = 128 points, din = 3):
  H1(i,j) = relu(W1A @ x_j + W1B @ x_i + b1)          (64)
  H2(i,j) = relu(W2 @ H1 + b2)                        (128)
  G(i,j)  = W3 @ H2                                    (256, b3 deferred)
  P       = max_{i,j} G + b3                           (256)
  Y       = V3 @ relu(V2 @ relu(V1 @ P + c1) + c2) + c3  (40)
  out[b]  = max_d Y[b, d]

Sharding: 16 (b, d) groups over 8 cores, 2 groups per core; host does the
final max over d.

Design (v4):
 * H1 is computed ON THE PE via selection-matrix matmuls
     pre_H1 = uT.T @ sel  +  v2T.T @ iden      (K=64 + K=128, accumulated)
   where sel = delta(jp == 4*it+q) and iden = I_128 tiled 4x along
   columns (both 0/1 host constants); b1 - b2 is folded into uT via an
   extra ones row of the x input.  This replaces 128 per-j-pair DVE/ACT
   elementwise builds (~35-48us) with ~14us of otherwise-idle PE time.
 * The h1 relu and the h2 relu run FUSED as one [1536]-col ACT
   activation (+b2 bias) per iteration: PSUM is laid out
   [preA 512 | l2 1024 | preB 512 | G0 1024 | G1 1024] so both pre-buf
   parities are contiguous with the l2 buffer; the baked-in -b2 on the
   pre region cancels the op's +b2 bias exactly.
 * L2 runs as two N=512 matmuls (one per weight half) -- the pair
   ordering inside l2ps is irrelevant because everything is max-pooled.
 * The two groups run lockstep (even iter -> group 0, odd -> group 1),
   4 j-pairs (1024 pairs) per iteration, with a software-pipeline skew:
   each epoch issues sel(i+2), fused-relu(i), L2(i+1), then L3(i-1) and
   its drains, so the latency-critical l2 recycle (relu -> L2 -> relu)
   never sits behind L3 on the in-order PE queue.
 * G drains are per-slot [1024] ops (pairing them into [2048] reduces
   re-serializes the G-slot recycle against L3 -- measured slower): most
   are direct DVE reduce_max from PSUM into per-group racc columns
   (fp32); 9 per group take the ACT-copy (fp16) + DVE tensor_tensor-max
   path to balance the engines, scheduled off the final iterations so
   the running-buffer reduction stays out of the tail.
 * F-MLP tail is batched across groups (N=2 matmuls) with c1/c2/c3
   folded in as K=1 matmul rows; a PE warmup burst at t=0 lifts the HAM
   clock gate to 2.4 GHz before the pipeline starts; input constants are
   packed into few DMAs (per-DMA boot latency dominates the prologue).
"""

import numpy as np
import ml_dtypes

import bass_rust
import concourse.bass as bass
import concourse.mybir as mybir
from concourse.tile import TileContext
from concourse.bass_utils import run_bass_kernel_spmd

BF16 = ml_dtypes.bfloat16
F32 = np.float32
DT = mybir.dt
ALU = mybir.AluOpType
AX = mybir.AxisListType
RELU = mybir.ActivationFunctionType.Relu

N_CORES = 8
B, N, DIN = 4, 512, 3
D = 4                    # boost factor
M = N // D               # 128 points per group
GROUPS_PER_CORE = 2
JP = M // 2              # 64 stacked j-pairs per group
NITER = 32               # lockstep iterations (16 per group, 4 jp each)
NWARM = 72               # warmup junk matmuls to lift the HAM clock gate
COPY_MOD = 4


def _is_copy(k):
    # per-group drain k: ACT-copy path (spread; none in the last iterations)
    return k % COPY_MOD == 1 or k == 14


N_COPY_PG = sum(1 for k in range(NITER) if _is_copy(k))
N_DIR_PG = NITER - N_COPY_PG


def _split_multi_waits(nc):
    """This walrus build accepts at most ONE sync wait per instruction;
    hoist extra waits onto same-engine nops inserted before the offender."""
    seq = 0
    for fn in nc.m.functions:
        for bb in fn.blocks:
            new = []
            changed = False
            for ins in bb.instructions:
                si = ins.sync_info
                waits = list(si.on_wait) if si is not None and si.on_wait else []
                if len(waits) > 1:
                    changed = True
                    for w in waits[:-1]:
                        seq += 1
                        new.append(
                            mybir.InstNoOp(
                                name=f"I-wsplit-{seq}",
                                engine=ins.engine,
                                sync_info=bass_rust.SyncInfo(
                                    on_wait=[w], on_update=[]
                                ),
                            )
                        )
                    ins.sync_info = bass_rust.SyncInfo(
                        on_wait=[waits[-1]], on_update=list(si.on_update or [])
                    )
                new.append(ins)
            if changed:
                bb.instructions = new


# ---------------------------------------------------------------------------
# Device program
# ---------------------------------------------------------------------------
def _build_program():
    nc = bass.Bass(
        "TRN2", target_bir_lowering=False, debug=False, num_devices=N_CORES
    )

    # x with an appended ones row (folds b1 into the uT prep matmul)
    xt = nc.declare_dram_parameter(
        "xt", [DIN + 1, GROUPS_PER_CORE, M], DT.bfloat16, isOutput=False
    )
    # cols 0:64 urhs_e0, 64:128 urhs_e1, 128:256 w1b2, 256:768 iden, 768:1280 wblob
    bigb = nc.declare_dram_parameter("bigb", [128, 1280], DT.bfloat16, isOutput=False)
    # sel[jp, it*512 + q*128 + i] = (jp == 4*it + q)
    selb = nc.declare_dram_parameter("selb", [JP, (NITER // 2) * 512], DT.bfloat16, isOutput=False)
    # v1t (2x512) | v2t (4x256) | v3t (2x40)
    vblob = nc.declare_dram_parameter("vblob", [128, 2938], DT.float16, isOutput=False)
    # col 1 b2c, 2:4 b3_2, 4:12 c1_42 (4mm x 2g), 12:16 c2_22, 16 c3
    cblob = nc.declare_dram_parameter("cblob", [128, 19], DT.float32, isOutput=False)
    y_out = nc.declare_dram_parameter(
        "y", [40, GROUPS_PER_CORE], DT.float32, isOutput=True
    )
    scr_out = nc.declare_dram_parameter("scr", [1, 16], DT.float32, isOutput=True)

    with TileContext(nc) as tc:
        with (
            tc.tile_pool(name="singles", bufs=1) as singles,
            tc.tile_pool(name="xtp", bufs=2) as xtp,
            tc.tile_pool(name="vup", bufs=8) as vup,
            tc.tile_pool(name="h1p", bufs=4) as h1pool,
            tc.tile_pool(name="h2p", bufs=5) as h2pool,
            tc.tile_pool(name="gcp", bufs=5) as gcpool,
            tc.tile_pool(name="fmlp", bufs=12) as fmlp,
            tc.tile_pool(name="psum", bufs=1, space="PSUM") as psum,
        ):
            # The whole PSUM as one tile.
            mega = psum.tile([128, 4096], DT.float32, tag="mega")

            # ---- input DMAs (sync + gpsimd queues; ACT/DVE stay free).
            # sel is 1MB: split into chunks so iter 0 isn't gated on it ----
            sb_xtall = xtp.tile([DIN + 1, GROUPS_PER_CORE, M], DT.bfloat16, tag="xtall")
            nc.sync.dma_start(out=sb_xtall, in_=xt[:, :, :])
            sb_xts = [sb_xtall[:, 0, :], sb_xtall[:, 1, :]]
            sb_big = singles.tile([128, 1280], DT.bfloat16, tag="bigb")
            nc.sync.dma_start(out=sb_big, in_=bigb[:, :])
            sb_sel = singles.tile([JP, (NITER // 2) * 512], DT.bfloat16, tag="selb")
            nc.sync.dma_start(out=sb_sel[:, 0:2048], in_=selb[:, 0:2048])
            nc.sync.dma_start(out=sb_sel[:, 2048:5120], in_=selb[:, 2048:5120])
            sb_c = singles.tile([128, 19], DT.float32, tag="cblob")
            nc.gpsimd.dma_start(out=sb_c, in_=cblob[:, :])
            nc.gpsimd.dma_start(out=sb_sel[:, 5120:8192], in_=selb[:, 5120:8192])
            sb_v = singles.tile([128, 2938], DT.float16, tag="vblob")
            nc.gpsimd.dma_start(out=sb_v, in_=vblob[:, :])
            sb_urhs0 = sb_big[0 : DIN + 1, 0:64]
            sb_urhs1 = sb_big[0 : DIN + 1, 64:128]
            sb_w1b2 = sb_big[0:DIN, 128:256]
            sb_iden = sb_big[:, 256:768]
            sb_w = sb_big[:, 768:1280]

            # ---- PE warmup burst (junk matmuls while DMAs land) ----
            wjunk = singles.tile([128, 128], DT.bfloat16, tag="wjunk")
            nc.vector.memset(wjunk, 0.0)
            for _ in range(NWARM):
                nc.tensor.matmul(
                    mega[:, 3200:3328], lhsT=wjunk, rhs=wjunk,
                    start=True, stop=True,
                )

            # dummy relu to hoist ACT_TABLE_LOAD into the init shadow
            warm = singles.tile([1, 1], DT.float32, tag="warm")
            nc.vector.memset(warm, 0.0)
            nc.scalar.activation(out=warm, in_=warm, func=RELU)

            sb_w2a, sb_w2b = sb_w[:, 0:128], sb_w[:, 128:256]
            sb_w3a, sb_w3b = sb_w[:, 256:384], sb_w[:, 384:512]
            sb_b2c = sb_c[:, 1:2]
            sb_b3_2 = sb_c[:, 2:4]
            sb_c1_42 = sb_c[:, 4:12]
            sb_c2_22 = sb_c[:, 12:16]
            sb_c3c = sb_c[0:40, 16:17]

            def v1t(k):  # [128, 512] fp16, k in 0..1
                return sb_v[:, 512 * k : 512 * (k + 1)]

            def v2t(k):  # [128, 256] fp16, k in 0..3
                return sb_v[:, 1024 + 256 * k : 1024 + 256 * (k + 1)]

            def v3t(k):  # [128, 40] fp16, k in 0..1
                return sb_v[:, 2048 + 40 * k : 2048 + 40 * (k + 1)]

            # ---- per-group prep: uT [64,128] and v2T [128,128] in SBUF bf16
            # (psum carved from the G regions, consumed before first L3) ----
            uT_sbs, v2T_sbs = [], []
            for g in range(GROUPS_PER_CORE):
                sb_xt = sb_xts[g]
                xt_eo = sb_xt.rearrange("k (j two) -> k two j", two=2)
                uTps = mega[0:JP, 2048 + 512 * g : 2048 + 512 * g + 128]
                nc.tensor.matmul(
                    uTps[:, 0:64], lhsT=xt_eo[:, 0, :], rhs=sb_urhs0,
                    start=True, stop=True,
                )
                nc.tensor.matmul(
                    uTps[:, 64:128], lhsT=xt_eo[:, 1, :], rhs=sb_urhs1,
                    start=True, stop=True,
                )
                uT_sb = vup.tile([JP, 128], DT.bfloat16, tag=f"uT{g}")
                nc.vector.tensor_copy(out=uT_sb, in_=uTps)
                v2Tps = mega[:, 3072 + 512 * g : 3072 + 512 * g + 128]
                nc.tensor.matmul(
                    v2Tps, lhsT=sb_xt[0:DIN, :], rhs=sb_w1b2,
                    start=True, stop=True,
                )
                v2T_sb = vup.tile([128, 128], DT.bfloat16, tag=f"v2T{g}")
                nc.vector.tensor_copy(out=v2T_sb, in_=v2Tps)
                uT_sbs.append(uT_sb)
                v2T_sbs.append(v2T_sb)

            # per-group accumulators
            raccs, rbs, rb_init = [], [], [False, False]
            for g in range(GROUPS_PER_CORE):
                racc = vup.tile([128, 2, N_DIR_PG], DT.float32, tag=f"racc{g}")
                raccs.append(racc)
                rb = vup.tile([128, 1024], DT.float16, tag=f"rb{g}")
                rbs.append(rb)
            dcount = [0, 0]
            pmBs = []
            for g in range(GROUPS_PER_CORE):
                pmB = fmlp.tile([128, 2], DT.float32, tag=f"pmB{g}")
                pmBs.append(pmB)
            pb = fmlp.tile([128, 2, 2], DT.float16, tag="pb")

            def issue_pm(g):
                pmA = fmlp.tile([128, 2], DT.float32, tag=f"pmA{g}")
                nc.vector.reduce_max(out=pmA, in_=raccs[g], axis=AX.X)
                pmx = fmlp.tile([128, 2], DT.float32, tag=f"pmx{g}")
                nc.vector.tensor_tensor(out=pmx, in0=pmA, in1=pmBs[g], op=ALU.max)
                nc.vector.tensor_tensor(
                    out=pb[:, :, g], in0=pmx, in1=sb_b3_2, op=ALU.add
                )

            l2ps = mega[:, 512:1536]

            def issue_sel(i):
                g, it = i % 2, i // 2
                pre = mega[:, 0:512] if i % 2 == 0 else mega[:, 1536:2048]
                nc.tensor.matmul(
                    pre, lhsT=uT_sbs[g],
                    rhs=sb_sel[:, 512 * it : 512 * (it + 1)],
                    start=True, stop=False,
                )
                nc.tensor.matmul(
                    pre, lhsT=v2T_sbs[g], rhs=sb_iden, start=False, stop=True,
                )
                return pre

            def issue_l2(h1):
                # pair order inside l2ps is irrelevant (max-pooled later):
                # one N=512 matmul per weight half
                nc.tensor.matmul(
                    l2ps[:, 0:512], lhsT=sb_w2a, rhs=h1[:, 0:512],
                    start=True, stop=True,
                )
                nc.tensor.matmul(
                    l2ps[:, 512:1024], lhsT=sb_w2b, rhs=h1[:, 0:512],
                    start=True, stop=True,
                )

            def issue_l3_drains(i, h2):
                g, it = i % 2, i // 2
                g0 = mega[:, 2048:3072]
                g1 = mega[:, 3072:4096]
                nc.tensor.matmul(
                    g0[:, 0:512], lhsT=sb_w3a, rhs=h2[:, 0:512],
                    start=True, stop=True,
                )
                nc.tensor.matmul(
                    g1[:, 0:512], lhsT=sb_w3a, rhs=h2[:, 512:1024],
                    start=True, stop=True,
                )
                nc.tensor.matmul(
                    g0[:, 512:1024], lhsT=sb_w3b, rhs=h2[:, 0:512],
                    start=True, stop=True,
                )
                nc.tensor.matmul(
                    g1[:, 512:1024], lhsT=sb_w3b, rhs=h2[:, 512:1024],
                    start=True, stop=True,
                )
                if i == 28:
                    # warm the output-DMA path for the final y DMA
                    nc.sync.dma_start(out=scr_out[:, :], in_=sb_c[0:1, 0:16])
                for sl, gp in enumerate((g0, g1)):
                    k = 2 * it + sl
                    if _is_copy(k):
                        gc = gcpool.tile([128, 1024], DT.float16)
                        nc.scalar.copy(out=gc, in_=gp)
                        if not rb_init[g]:
                            rb_init[g] = True
                            nc.vector.tensor_copy(out=rbs[g], in_=gc)
                        else:
                            nc.vector.tensor_tensor(
                                out=rbs[g], in0=gc, in1=rbs[g], op=ALU.max
                            )
                        if k == 29:
                            nc.vector.reduce_max(
                                out=pmBs[g],
                                in_=rbs[g].rearrange("p (a b) -> p a b", a=2),
                                axis=AX.X,
                            )
                    else:
                        t = dcount[g]
                        dcount[g] += 1
                        nc.vector.reduce_max(
                            out=raccs[g][:, :, t : t + 1],
                            in_=gp.rearrange("p (a b) -> p a b", a=2),
                            axis=AX.X,
                        )

            # ---- main lockstep pipeline: fused [preH1|l2] relu, L3 lagged
            # one epoch so the ACT->PE->ACT recycle stays short ----
            h1s, h2s = {}, {}
            pre0 = issue_sel(0)
            h1t0 = h1pool.tile([128, 512], DT.bfloat16)
            nc.scalar.activation(out=h1t0, in_=pre0, func=RELU, bias=sb_b2c, scale=1.0)
            h1s[0] = h1t0
            issue_l2(h1s[0])
            pre1 = issue_sel(1)
            h1t1 = h1pool.tile([128, 512], DT.bfloat16)
            nc.scalar.activation(out=h1t1, in_=pre1, func=RELU, bias=sb_b2c, scale=1.0)
            h1s[1] = h1t1

            for i in range(NITER):
                if i + 2 < NITER:
                    issue_sel(i + 2)
                    combo = h2pool.tile([128, 1536], DT.bfloat16)
                    if i % 2 == 0:
                        nc.scalar.activation(
                            out=combo, in_=mega[:, 0:1536], func=RELU,
                            bias=sb_b2c, scale=1.0,
                        )
                        h1s[i + 2] = combo[:, 0:512]
                        h2s[i] = combo[:, 512:1536]
                    else:
                        nc.scalar.activation(
                            out=combo, in_=mega[:, 512:2048], func=RELU,
                            bias=sb_b2c, scale=1.0,
                        )
                        h2s[i] = combo[:, 0:1024]
                        h1s[i + 2] = combo[:, 1024:1536]
                else:
                    combo = h2pool.tile([128, 1536], DT.bfloat16)
                    nc.scalar.activation(
                        out=combo[:, 0:1024], in_=l2ps, func=RELU,
                        bias=sb_b2c, scale=1.0,
                    )
                    h2s[i] = combo[:, 0:1024]
                if i + 1 < NITER:
                    issue_l2(h1s[i + 1])
                if i >= 1:
                    issue_l3_drains(i - 1, h2s[i - 1])
                    if i == NITER - 1:
                        issue_pm(0)
            issue_l3_drains(NITER - 1, h2s[NITER - 1])
            issue_pm(1)

            # ---- batched F-MLP (N=2); pb written by issue_pm above ----
            ones2 = sb_v[0:1, 2936:2938]
            y1ps = mega[:, 0:8].rearrange("p (m g) -> p m g", m=4)
            for mm in range(4):
                for kk in range(2):
                    nc.tensor.matmul(
                        y1ps[:, mm, :],
                        lhsT=v1t(kk)[:, mm * 128 : (mm + 1) * 128],
                        rhs=pb[:, kk, :],
                        start=(kk == 0),
                        stop=False,
                    )
                nc.tensor.matmul(
                    y1ps[:, mm, :],
                    lhsT=sb_v[0:1, 2128 + mm * 128 : 2128 + (mm + 1) * 128],
                    rhs=ones2,
                    start=False, stop=True,
                )
            y1 = fmlp.tile([128, 4, 2], DT.float16, tag="y1")
            nc.vector.tensor_scalar_max(
                out=y1.rearrange("p m g -> p (m g)"), in0=mega[:, 0:8],
                scalar1=0.0,
            )

            y2ps = mega[:, 1024:1028].rearrange("p (m g) -> p m g", m=2)
            for mm in range(2):
                for kk in range(4):
                    nc.tensor.matmul(
                        y2ps[:, mm, :],
                        lhsT=v2t(kk)[:, mm * 128 : (mm + 1) * 128],
                        rhs=y1[:, kk, :],
                        start=(kk == 0),
                        stop=False,
                    )
                nc.tensor.matmul(
                    y2ps[:, mm, :],
                    lhsT=sb_v[0:1, 2640 + mm * 128 : 2640 + (mm + 1) * 128],
                    rhs=ones2,
                    start=False, stop=True,
                )
            y2 = fmlp.tile([128, 2, 2], DT.float16, tag="y2")
            nc.vector.tensor_scalar_max(
                out=y2.rearrange("p m g -> p (m g)"), in0=mega[:, 1024:1028],
                scalar1=0.0,
            )

            y3ps = mega[0:40, 2048:2050]
            for kk in range(2):
                nc.tensor.matmul(
                    y3ps,
                    lhsT=v3t(kk)[:, 0:40],
                    rhs=y2[:, kk, :],
                    start=(kk == 0),
                    stop=False,
                )
            nc.tensor.matmul(
                y3ps, lhsT=sb_v[0:1, 2896:2936], rhs=ones2,
                start=False, stop=True,
            )
            y3 = fmlp.tile([40, 2], DT.float32, tag="y3")
            nc.vector.tensor_copy(out=y3, in_=y3ps)
            nc.gpsimd.dma_start(out=y_out[:, :], in_=y3)

    _split_multi_waits(nc)
    return nc


# ---------------------------------------------------------------------------
# Host side
# ---------------------------------------------------------------------------
_NC_CACHE = None


def _get_program():
    global _NC_CACHE
    if _NC_CACHE is None:
        _NC_CACHE = _build_program()
    return _NC_CACHE


def _make_in_maps(inputs):
    X = np.asarray(inputs["X"], F32)
    W1 = np.asarray(inputs["W1"], F32)
    b1 = np.asarray(inputs["b1"], F32)
    W2 = np.asarray(inputs["W2"], F32)
    b2 = np.asarray(inputs["b2"], F32)
    W3 = np.asarray(inputs["W3"], F32)
    b3 = np.asarray(inputs["b3"], F32)
    V1 = np.asarray(inputs["V1"], F32)
    c1 = np.asarray(inputs["c1"], F32)
    V2 = np.asarray(inputs["V2"], F32)
    c2 = np.asarray(inputs["c2"], F32)
    V3 = np.asarray(inputs["V3"], F32)
    c3 = np.asarray(inputs["c3"], F32)

    W1A, W1B = W1[:, :DIN], W1[:, DIN:]
    # sel[jp, it*512 + q*128 + i] = (jp == 4*it + q)
    selblob = np.zeros((JP, (NITER // 2) * 512), F32)
    for jp in range(JP):
        it, q = jp // 4, jp % 4
        selblob[jp, it * 512 + q * 128 : it * 512 + (q + 1) * 128] = 1.0
    selblob = selblob.astype(BF16)

    z64 = np.zeros((64, 128), F32)
    bigblob = np.zeros((128, 1280), F32)
    # u-rhs halves: ones-row contributes b1 - b2[half] (cancels the fused
    # relu's +b2 bias on the preH1 region)
    bigblob[0:DIN, 0:64] = W1A.T
    bigblob[DIN, 0:64] = b1 - b2[0:64]
    bigblob[0:DIN, 64:128] = W1A.T
    bigblob[DIN, 64:128] = b1 - b2[64:128]
    bigblob[0:DIN, 128:256] = np.concatenate([W1B.T, W1B.T], axis=1)
    # iden region
    bigblob[:, 256:768] = np.tile(np.eye(M, dtype=F32), (1, 4))
    # wblob region
    bigblob[:, 768:1280] = np.concatenate(
        [
            np.concatenate([W2.T, z64], axis=0),
            np.concatenate([z64, W2.T], axis=0),
            W3.T[:, 0:128],
            W3.T[:, 128:256],
        ],
        axis=1,
    )
    bigblob = bigblob.astype(BF16)
    v1t_cols = V1.T.reshape(2, 128, 512).transpose(1, 0, 2).reshape(128, 1024)
    crows = np.zeros((128, 810), F32)
    crows[0, 0:512] = c1
    crows[0, 512:768] = c2
    crows[0, 768:808] = c3
    crows[0, 808:810] = 1.0
    vblob = np.concatenate(
        [v1t_cols,
         V2.T.reshape(4, 128, 256).transpose(1, 0, 2).reshape(128, 1024),
         V3.T.reshape(2, 128, 40).transpose(1, 0, 2).reshape(128, 80),
         crows],
        axis=1,
    ).astype(np.float16)
    cblob = np.zeros((128, 19), F32)
    cblob[:, 17:19] = 1.0
    cblob[:, 1] = b2
    cblob[:, 2:4] = b3.reshape(2, 128).T
    cblob[:, 4:12] = np.repeat(c1.reshape(4, 128).T, 2, axis=1)
    cblob[:, 12:16] = np.repeat(c2.reshape(2, 128).T, 2, axis=1)
    cblob[0:40, 16] = c3

    shared = dict(
        bigb=bigblob, vblob=vblob, cblob=cblob, selb=selblob,
    )

    Xv = X.reshape(B, D, M, DIN)
    in_maps = []
    for c in range(N_CORES):
        xts = np.ones((DIN + 1, GROUPS_PER_CORE, M), F32)
        for gi in range(GROUPS_PER_CORE):
            g = 2 * c + gi
            bb, dd = g // D, g % D
            xts[0:DIN, gi] = Xv[bb, dd].T
        in_maps.append(dict(shared, xt=xts.astype(BF16)))
    return in_maps


def _run(inputs, trace=False):
    nc = _get_program()
    in_maps = _make_in_maps(inputs)
    res = run_bass_kernel_spmd(nc, in_maps, list(range(N_CORES)), trace=trace)
    ys = np.stack([res.results[c]["y"].T for c in range(N_CORES)])  # [8, 2, 40]
    y16 = ys.reshape(B, D, 40)
    out = y16.max(axis=1).astype(F32)
    return out, res


def kernel(**inputs):
    out, _ = _run(inputs, trace=False)
    return out


# revision 45
# speedup vs baseline: 1.0239x; 1.0179x over previous
"""Trainium2 Bass kernel for BoostedPointPairNet2.

Model (per (b, d) group, m = 128 points, din = 3):
  H1(i,j) = relu(W1A @ x_j + W1B @ x_i + b1)          (64)
  H2(i,j) = relu(W2 @ H1 + b2)                        (128)
  G(i,j)  = W3 @ H2                                    (256, b3 deferred)
  P       = max_{i,j} G + b3                           (256)
  Y       = V3 @ relu(V2 @ relu(V1 @ P + c1) + c2) + c3  (40)
  out[b]  = max_d Y[b, d]

Sharding: 16 (b, d) groups over 8 cores, 2 groups per core; host does the
final max over d.

Design (v4):
 * H1 is computed ON THE PE via selection-matrix matmuls
     pre_H1 = uT.T @ sel  +  v2T.T @ iden      (K=64 + K=128, accumulated)
   where sel = delta(jp == 4*it+q) and iden = I_128 tiled 4x along
   columns (both 0/1 host constants); b1 - b2 is folded into uT via an
   extra ones row of the x input.  This replaces 128 per-j-pair DVE/ACT
   elementwise builds (~35-48us) with ~14us of otherwise-idle PE time.
 * The h1 relu and the h2 relu run FUSED as one [1536]-col ACT
   activation (+b2 bias) per iteration: PSUM is laid out
   [preA 512 | l2 1024 | preB 512 | G0 1024 | G1 1024] so both pre-buf
   parities are contiguous with the l2 buffer; the baked-in -b2 on the
   pre region cancels the op's +b2 bias exactly.
 * L2 runs as two N=512 matmuls (one per weight half) -- the pair
   ordering inside l2ps is irrelevant because everything is max-pooled.
 * The two groups run lockstep (even iter -> group 0, odd -> group 1),
   4 j-pairs (1024 pairs) per iteration, with a software-pipeline skew:
   each epoch issues sel(i+2), fused-relu(i), L2(i+1), then L3(i-1) and
   its drains, so the latency-critical l2 recycle (relu -> L2 -> relu)
   never sits behind L3 on the in-order PE queue.
 * G drains are per-slot [1024] ops (pairing them into [2048] reduces
   re-serializes the G-slot recycle against L3 -- measured slower): most
   are direct DVE reduce_max from PSUM into per-group racc columns
   (fp32); 9 per group take the ACT-copy (fp16) + DVE tensor_tensor-max
   path to balance the engines, scheduled off the final iterations so
   the running-buffer reduction stays out of the tail.
 * F-MLP tail is batched across groups (N=2 matmuls) with c1/c2/c3
   folded in as K=1 matmul rows; a PE warmup burst at t=0 lifts the HAM
   clock gate to 2.4 GHz before the pipeline starts; input constants are
   packed into few DMAs (per-DMA boot latency dominates the prologue).
"""

import numpy as np
import ml_dtypes

import bass_rust
import concourse.bass as bass
import concourse.mybir as mybir
from concourse.tile import TileContext
from concourse.bass_utils import run_bass_kernel_spmd

BF16 = ml_dtypes.bfloat16
F32 = np.float32
DT = mybir.dt
ALU = mybir.AluOpType
AX = mybir.AxisListType
RELU = mybir.ActivationFunctionType.Relu

N_CORES = 8
B, N, DIN = 4, 512, 3
D = 4                    # boost factor
M = N // D               # 128 points per group
GROUPS_PER_CORE = 2
JP = M // 2              # 64 stacked j-pairs per group
NITER = 32               # lockstep iterations (16 per group, 4 jp each)
NWARM = 72               # warmup junk matmuls to lift the HAM clock gate
COPY_MOD = 4


def _is_copy(k):
    # per-group drain k: ACT-copy path (spread; none in the last iterations)
    return k % COPY_MOD == 1 or k == 14


N_COPY_PG = sum(1 for k in range(NITER) if _is_copy(k))
N_DIR_PG = NITER - N_COPY_PG


def _split_multi_waits(nc):
    """This walrus build accepts at most ONE sync wait per instruction;
    hoist extra waits onto same-engine nops inserted before the offender."""
    seq = 0
    for fn in nc.m.functions:
        for bb in fn.blocks:
            new = []
            changed = False
            for ins in bb.instructions:
                si = ins.sync_info
                waits = list(si.on_wait) if si is not None and si.on_wait else []
                if len(waits) > 1:
                    changed = True
                    for w in waits[:-1]:
                        seq += 1
                        new.append(
                            mybir.InstNoOp(
                                name=f"I-wsplit-{seq}",
                                engine=ins.engine,
                                sync_info=bass_rust.SyncInfo(
                                    on_wait=[w], on_update=[]
                                ),
                            )
                        )
                    ins.sync_info = bass_rust.SyncInfo(
                        on_wait=[waits[-1]], on_update=list(si.on_update or [])
                    )
                new.append(ins)
            if changed:
                bb.instructions = new


# ---------------------------------------------------------------------------
# Device program
# ---------------------------------------------------------------------------
def _build_program():
    nc = bass.Bass(
        "TRN2", target_bir_lowering=False, debug=False, num_devices=N_CORES
    )

    # x with an appended ones row (folds b1 into the uT prep matmul)
    xt = nc.declare_dram_parameter(
        "xt", [DIN + 1, GROUPS_PER_CORE, M], DT.bfloat16, isOutput=False
    )
    # cols 0:64 urhs_e0, 64:128 urhs_e1, 128:256 w1b2, 256:768 iden, 768:1280 wblob
    bigb = nc.declare_dram_parameter("bigb", [128, 1280], DT.bfloat16, isOutput=False)
    # sel[jp, it*512 + q*128 + i] = (jp == 4*it + q)
    selb = nc.declare_dram_parameter("selb", [JP, (NITER // 2) * 512], DT.bfloat16, isOutput=False)
    # v1t (2x512) | v2t (4x256) | v3t (2x40)
    vblob = nc.declare_dram_parameter("vblob", [128, 2938], DT.float16, isOutput=False)
    # col 1 b2c, 2:4 b3_2, 4:12 c1_42 (4mm x 2g), 12:16 c2_22, 16 c3
    cblob = nc.declare_dram_parameter("cblob", [128, 19], DT.float32, isOutput=False)
    y_out = nc.declare_dram_parameter(
        "y", [40, GROUPS_PER_CORE], DT.float32, isOutput=True
    )
    scr_out = nc.declare_dram_parameter("scr", [1, 16], DT.float32, isOutput=True)

    with TileContext(nc) as tc:
        with (
            tc.tile_pool(name="singles", bufs=1) as singles,
            tc.tile_pool(name="xtp", bufs=2) as xtp,
            tc.tile_pool(name="vup", bufs=8) as vup,
            tc.tile_pool(name="h1p", bufs=4) as h1pool,
            tc.tile_pool(name="h2p", bufs=5) as h2pool,
            tc.tile_pool(name="gcp", bufs=5) as gcpool,
            tc.tile_pool(name="fmlp", bufs=12) as fmlp,
            tc.tile_pool(name="psum", bufs=1, space="PSUM") as psum,
        ):
            # The whole PSUM as one tile.
            mega = psum.tile([128, 4096], DT.float32, tag="mega")

            # ---- input DMAs (sync + gpsimd queues; ACT/DVE stay free).
            # sel is 1MB: split into chunks so iter 0 isn't gated on it ----
            sb_xtall = xtp.tile([DIN + 1, GROUPS_PER_CORE, M], DT.bfloat16, tag="xtall")
            nc.sync.dma_start(out=sb_xtall, in_=xt[:, :, :])
            sb_xts = [sb_xtall[:, 0, :], sb_xtall[:, 1, :]]
            sb_big = singles.tile([128, 1280], DT.bfloat16, tag="bigb")
            nc.sync.dma_start(out=sb_big, in_=bigb[:, :])
            sb_sel = singles.tile([JP, (NITER // 2) * 512], DT.bfloat16, tag="selb")
            nc.sync.dma_start(out=sb_sel[:, 0:2048], in_=selb[:, 0:2048])
            nc.sync.dma_start(out=sb_sel[:, 2048:5120], in_=selb[:, 2048:5120])
            sb_c = singles.tile([128, 19], DT.float32, tag="cblob")
            nc.gpsimd.dma_start(out=sb_c, in_=cblob[:, :])
            nc.gpsimd.dma_start(out=sb_sel[:, 5120:8192], in_=selb[:, 5120:8192])
            sb_v = singles.tile([128, 2938], DT.float16, tag="vblob")
            nc.gpsimd.dma_start(out=sb_v, in_=vblob[:, :])
            sb_urhs0 = sb_big[0 : DIN + 1, 0:64]
            sb_urhs1 = sb_big[0 : DIN + 1, 64:128]
            sb_w1b2 = sb_big[0:DIN, 128:256]
            sb_iden = sb_big[:, 256:768]
            sb_w = sb_big[:, 768:1280]

            # ---- PE warmup burst (junk matmuls while DMAs land) ----
            wjunk = singles.tile([128, 128], DT.bfloat16, tag="wjunk")
            nc.vector.memset(wjunk, 0.0)
            for _ in range(NWARM):
                nc.tensor.matmul(
                    mega[:, 3200:3328], lhsT=wjunk, rhs=wjunk,
                    start=True, stop=True,
                )

            # dummy relu to hoist ACT_TABLE_LOAD into the init shadow
            warm = singles.tile([1, 1], DT.float32, tag="warm")
            nc.vector.memset(warm, 0.0)
            nc.scalar.activation(out=warm, in_=warm, func=RELU)

            sb_w2a, sb_w2b = sb_w[:, 0:128], sb_w[:, 128:256]
            sb_w3a, sb_w3b = sb_w[:, 256:384], sb_w[:, 384:512]
            sb_b2c = sb_c[:, 1:2]
            sb_b3_2 = sb_c[:, 2:4]
            sb_c1_42 = sb_c[:, 4:12]
            sb_c2_22 = sb_c[:, 12:16]
            sb_c3c = sb_c[0:40, 16:17]

            def v1t(k):  # [128, 512] fp16, k in 0..1
                return sb_v[:, 512 * k : 512 * (k + 1)]

            def v2t(k):  # [128, 256] fp16, k in 0..3
                return sb_v[:, 1024 + 256 * k : 1024 + 256 * (k + 1)]

            def v3t(k):  # [128, 40] fp16, k in 0..1
                return sb_v[:, 2048 + 40 * k : 2048 + 40 * (k + 1)]

            # ---- per-group prep: uT [64,128] and v2T [128,128] in SBUF bf16
            # (psum carved from the G regions, consumed before first L3) ----
            uT_sbs, v2T_sbs = [], []
            for g in range(GROUPS_PER_CORE):
                sb_xt = sb_xts[g]
                xt_eo = sb_xt.rearrange("k (j two) -> k two j", two=2)
                uTps = mega[0:JP, 2048 + 512 * g : 2048 + 512 * g + 128]
                nc.tensor.matmul(
                    uTps[:, 0:64], lhsT=xt_eo[:, 0, :], rhs=sb_urhs0,
                    start=True, stop=True,
                )
                nc.tensor.matmul(
                    uTps[:, 64:128], lhsT=xt_eo[:, 1, :], rhs=sb_urhs1,
                    start=True, stop=True,
                )
                uT_sb = vup.tile([JP, 128], DT.bfloat16, tag=f"uT{g}")
                nc.vector.tensor_copy(out=uT_sb, in_=uTps)
                v2Tps = mega[:, 3072 + 512 * g : 3072 + 512 * g + 128]
                nc.tensor.matmul(
                    v2Tps, lhsT=sb_xt[0:DIN, :], rhs=sb_w1b2,
                    start=True, stop=True,
                )
                v2T_sb = vup.tile([128, 128], DT.bfloat16, tag=f"v2T{g}")
                nc.vector.tensor_copy(out=v2T_sb, in_=v2Tps)
                uT_sbs.append(uT_sb)
                v2T_sbs.append(v2T_sb)

            # per-group accumulators
            raccs, rbs, rb_init = [], [], [False, False]
            for g in range(GROUPS_PER_CORE):
                racc = vup.tile([128, 2, N_DIR_PG], DT.float32, tag=f"racc{g}")
                raccs.append(racc)
                rb = vup.tile([128, 1024], DT.float16, tag=f"rb{g}")
                rbs.append(rb)
            dcount = [0, 0]
            pmBs = []
            for g in range(GROUPS_PER_CORE):
                pmB = fmlp.tile([128, 2], DT.float32, tag=f"pmB{g}")
                pmBs.append(pmB)

            l2ps = mega[:, 512:1536]

            def issue_sel(i):
                g, it = i % 2, i // 2
                pre = mega[:, 0:512] if i % 2 == 0 else mega[:, 1536:2048]
                nc.tensor.matmul(
                    pre, lhsT=uT_sbs[g],
                    rhs=sb_sel[:, 512 * it : 512 * (it + 1)],
                    start=True, stop=False,
                )
                nc.tensor.matmul(
                    pre, lhsT=v2T_sbs[g], rhs=sb_iden, start=False, stop=True,
                )
                return pre

            def issue_l2(h1):
                # pair order inside l2ps is irrelevant (max-pooled later):
                # one N=512 matmul per weight half
                nc.tensor.matmul(
                    l2ps[:, 0:512], lhsT=sb_w2a, rhs=h1[:, 0:512],
                    start=True, stop=True,
                )
                nc.tensor.matmul(
                    l2ps[:, 512:1024], lhsT=sb_w2b, rhs=h1[:, 0:512],
                    start=True, stop=True,
                )

            def issue_l3_drains(i, h2):
                g, it = i % 2, i // 2
                g0 = mega[:, 2048:3072]
                g1 = mega[:, 3072:4096]
                nc.tensor.matmul(
                    g0[:, 0:512], lhsT=sb_w3a, rhs=h2[:, 0:512],
                    start=True, stop=True,
                )
                nc.tensor.matmul(
                    g1[:, 0:512], lhsT=sb_w3a, rhs=h2[:, 512:1024],
                    start=True, stop=True,
                )
                nc.tensor.matmul(
                    g0[:, 512:1024], lhsT=sb_w3b, rhs=h2[:, 0:512],
                    start=True, stop=True,
                )
                nc.tensor.matmul(
                    g1[:, 512:1024], lhsT=sb_w3b, rhs=h2[:, 512:1024],
                    start=True, stop=True,
                )
                if i == 28:
                    # warm the output-DMA path for the final y DMA
                    nc.sync.dma_start(out=scr_out[:, :], in_=sb_c[0:1, 0:16])
                for sl, gp in enumerate((g0, g1)):
                    k = 2 * it + sl
                    if _is_copy(k):
                        gc = gcpool.tile([128, 1024], DT.float16)
                        nc.scalar.copy(out=gc, in_=gp)
                        if not rb_init[g]:
                            rb_init[g] = True
                            nc.vector.tensor_copy(out=rbs[g], in_=gc)
                        else:
                            nc.vector.tensor_tensor(
                                out=rbs[g], in0=gc, in1=rbs[g], op=ALU.max
                            )
                        if k == 29:
                            nc.vector.reduce_max(
                                out=pmBs[g],
                                in_=rbs[g].rearrange("p (a b) -> p a b", a=2),
                                axis=AX.X,
                            )
                    else:
                        t = dcount[g]
                        dcount[g] += 1
                        nc.vector.reduce_max(
                            out=raccs[g][:, :, t : t + 1],
                            in_=gp.rearrange("p (a b) -> p a b", a=2),
                            axis=AX.X,
                        )

            # ---- main lockstep pipeline: fused [preH1|l2] relu, L3 lagged
            # one epoch so the ACT->PE->ACT recycle stays short ----
            h1s, h2s = {}, {}
            pre0 = issue_sel(0)
            h1t0 = h1pool.tile([128, 512], DT.bfloat16)
            nc.scalar.activation(out=h1t0, in_=pre0, func=RELU, bias=sb_b2c, scale=1.0)
            h1s[0] = h1t0
            issue_l2(h1s[0])
            pre1 = issue_sel(1)
            h1t1 = h1pool.tile([128, 512], DT.bfloat16)
            nc.scalar.activation(out=h1t1, in_=pre1, func=RELU, bias=sb_b2c, scale=1.0)
            h1s[1] = h1t1

            for i in range(NITER):
                if i + 2 < NITER:
                    issue_sel(i + 2)
                    combo = h2pool.tile([128, 1536], DT.bfloat16)
                    if i % 2 == 0:
                        nc.scalar.activation(
                            out=combo, in_=mega[:, 0:1536], func=RELU,
                            bias=sb_b2c, scale=1.0,
                        )
                        h1s[i + 2] = combo[:, 0:512]
                        h2s[i] = combo[:, 512:1536]
                    else:
                        nc.scalar.activation(
                            out=combo, in_=mega[:, 512:2048], func=RELU,
                            bias=sb_b2c, scale=1.0,
                        )
                        h2s[i] = combo[:, 0:1024]
                        h1s[i + 2] = combo[:, 1024:1536]
                else:
                    combo = h2pool.tile([128, 1536], DT.bfloat16)
                    nc.scalar.activation(
                        out=combo[:, 0:1024], in_=l2ps, func=RELU,
                        bias=sb_b2c, scale=1.0,
                    )
                    h2s[i] = combo[:, 0:1024]
                if i + 1 < NITER:
                    issue_l2(h1s[i + 1])
                if i >= 1:
                    issue_l3_drains(i - 1, h2s[i - 1])
            issue_l3_drains(NITER - 1, h2s[NITER - 1])

            # ---- P per group, batched F-MLP (N=2); pb is (half, group) ----
            pb = fmlp.tile([128, 2, 2], DT.float16, tag="pb")
            for g in range(GROUPS_PER_CORE):
                pmA = fmlp.tile([128, 2], DT.float32, tag=f"pmA{g}")
                nc.vector.reduce_max(out=pmA, in_=raccs[g], axis=AX.X)
                pmx = fmlp.tile([128, 2], DT.float32, tag=f"pmx{g}")
                nc.vector.tensor_tensor(out=pmx, in0=pmA, in1=pmBs[g], op=ALU.max)
                nc.vector.tensor_tensor(
                    out=pb[:, :, g], in0=pmx, in1=sb_b3_2, op=ALU.add
                )

            ones2 = sb_v[0:1, 2936:2938]
            y1ps = mega[:, 0:8].rearrange("p (m g) -> p m g", m=4)
            for mm in range(4):
                for kk in range(2):
                    nc.tensor.matmul(
                        y1ps[:, mm, :],
                        lhsT=v1t(kk)[:, mm * 128 : (mm + 1) * 128],
                        rhs=pb[:, kk, :],
                        start=(kk == 0),
                        stop=False,
                    )
                nc.tensor.matmul(
                    y1ps[:, mm, :],
                    lhsT=sb_v[0:1, 2128 + mm * 128 : 2128 + (mm + 1) * 128],
                    rhs=ones2,
                    start=False, stop=True,
                )
            y1 = fmlp.tile([128, 4, 2], DT.float16, tag="y1")
            nc.vector.tensor_scalar_max(
                out=y1.rearrange("p m g -> p (m g)"), in0=mega[:, 0:8],
                scalar1=0.0,
            )

            y2ps = mega[:, 1024:1028].rearrange("p (m g) -> p m g", m=2)
            for mm in range(2):
                for kk in range(4):
                    nc.tensor.matmul(
                        y2ps[:, mm, :],
                        lhsT=v2t(kk)[:, mm * 128 : (mm + 1) * 128],
                        rhs=y1[:, kk, :],
                        start=(kk == 0),
                        stop=False,
                    )
                nc.tensor.matmul(
                    y2ps[:, mm, :],
                    lhsT=sb_v[0:1, 2640 + mm * 128 : 2640 + (mm + 1) * 128],
                    rhs=ones2,
                    start=False, stop=True,
                )
            y2 = fmlp.tile([128, 2, 2], DT.float16, tag="y2")
            nc.vector.tensor_scalar_max(
                out=y2.rearrange("p m g -> p (m g)"), in0=mega[:, 1024:1028],
                scalar1=0.0,
            )

            y3ps = mega[0:40, 2048:2050]
            for kk in range(2):
                nc.tensor.matmul(
                    y3ps,
                    lhsT=v3t(kk)[:, 0:40],
                    rhs=y2[:, kk, :],
                    start=(kk == 0),
                    stop=False,
                )
            nc.tensor.matmul(
                y3ps, lhsT=sb_v[0:1, 2896:2936], rhs=ones2,
                start=False, stop=True,
            )
            y3 = fmlp.tile([40, 2], DT.float32, tag="y3")
            nc.vector.tensor_copy(out=y3, in_=y3ps)
            nc.gpsimd.dma_start(out=y_out[:, :], in_=y3)

    _split_multi_waits(nc)
    return nc


# ---------------------------------------------------------------------------
# Host side
# ---------------------------------------------------------------------------
_NC_CACHE = None


def _get_program():
    global _NC_CACHE
    if _NC_CACHE is None:
        _NC_CACHE = _build_program()
    return _NC_CACHE


def _make_in_maps(inputs):
    X = np.asarray(inputs["X"], F32)
    W1 = np.asarray(inputs["W1"], F32)
    b1 = np.asarray(inputs["b1"], F32)
    W2 = np.asarray(inputs["W2"], F32)
    b2 = np.asarray(inputs["b2"], F32)
    W3 = np.asarray(inputs["W3"], F32)
    b3 = np.asarray(inputs["b3"], F32)
    V1 = np.asarray(inputs["V1"], F32)
    c1 = np.asarray(inputs["c1"], F32)
    V2 = np.asarray(inputs["V2"], F32)
    c2 = np.asarray(inputs["c2"], F32)
    V3 = np.asarray(inputs["V3"], F32)
    c3 = np.asarray(inputs["c3"], F32)

    W1A, W1B = W1[:, :DIN], W1[:, DIN:]
    # sel[jp, it*512 + q*128 + i] = (jp == 4*it + q)
    selblob = np.zeros((JP, (NITER // 2) * 512), F32)
    for jp in range(JP):
        it, q = jp // 4, jp % 4
        selblob[jp, it * 512 + q * 128 : it * 512 + (q + 1) * 128] = 1.0
    selblob = selblob.astype(BF16)

    z64 = np.zeros((64, 128), F32)
    bigblob = np.zeros((128, 1280), F32)
    # u-rhs halves: ones-row contributes b1 - b2[half] (cancels the fused
    # relu's +b2 bias on the preH1 region)
    bigblob[0:DIN, 0:64] = W1A.T
    bigblob[DIN, 0:64] = b1 - b2[0:64]
    bigblob[0:DIN, 64:128] = W1A.T
    bigblob[DIN, 64:128] = b1 - b2[64:128]
    bigblob[0:DIN, 128:256] = np.concatenate([W1B.T, W1B.T], axis=1)
    # iden region
    bigblob[:, 256:768] = np.tile(np.eye(M, dtype=F32), (1, 4))
    # wblob region
    bigblob[:, 768:1280] = np.concatenate(
        [
            np.concatenate([W2.T, z64], axis=0),
            np.concatenate([z64, W2.T], axis=0),
            W3.T[:, 0:128],
            W3.T[:, 128:256],
        ],
        axis=1,
    )
    bigblob = bigblob.astype(BF16)
    v1t_cols = V1.T.reshape(2, 128, 512).transpose(1, 0, 2).reshape(128, 1024)
    crows = np.zeros((128, 810), F32)
    crows[0, 0:512] = c1
    crows[0, 512:768] = c2
    crows[0, 768:808] = c3
    crows[0, 808:810] = 1.0
    vblob = np.concatenate(
        [v1t_cols,
         V2.T.reshape(4, 128, 256).transpose(1, 0, 2).reshape(128, 1024),
         V3.T.reshape(2, 128, 40).transpose(1, 0, 2).reshape(128, 80),
         crows],
        axis=1,
    ).astype(np.float16)
    cblob = np.zeros((128, 19), F32)
    cblob[:, 17:19] = 1.0
    cblob[:, 1] = b2
    cblob[:, 2:4] = b3.reshape(2, 128).T
    cblob[:, 4:12] = np.repeat(c1.reshape(4, 128).T, 2, axis=1)
    cblob[:, 12:16] = np.repeat(c2.reshape(2, 128).T, 2, axis=1)
    cblob[0:40, 16] = c3

    shared = dict(
        bigb=bigblob, vblob=vblob, cblob=cblob, selb=selblob,
    )

    Xv = X.reshape(B, D, M, DIN)
    in_maps = []
    for c in range(N_CORES):
        xts = np.ones((DIN + 1, GROUPS_PER_CORE, M), F32)
        for gi in range(GROUPS_PER_CORE):
            g = 2 * c + gi
            bb, dd = g // D, g % D
            xts[0:DIN, gi] = Xv[bb, dd].T
        in_maps.append(dict(shared, xt=xts.astype(BF16)))
    return in_maps


def _run(inputs, trace=False):
    nc = _get_program()
    in_maps = _make_in_maps(inputs)
    res = run_bass_kernel_spmd(nc, in_maps, list(range(N_CORES)), trace=trace)
    ys = np.stack([res.results[c]["y"].T for c in range(N_CORES)])  # [8, 2, 40]
    y16 = ys.reshape(B, D, 40)
    out = y16.max(axis=1).astype(F32)
    return out, res


def kernel(**inputs):
    out, _ = _run(inputs, trace=False)
    return out
